# revision 1
# baseline (speedup 1.0000x reference)
"""Trainium2 Bass kernel for nn_Blur1: 3x3 cross blur + LIF neuron scan.

Reference semantics (per timestep t, state v/i per pixel):
    c    = conv2d_same(x[t], K)        # K = cross kernel (0.15 sides, 0.4 ctr)
    v_d  = 0.8*v + 0.2*i
    z[t] = (v_d - 1) > 0
    v    = (1-z)*v_d
    i    = 0.8*i + c

Strategy (8 NeuronCores = 4 H-shards x 2 W-shards, no collectives; halos are
baked into the per-core input slices on the host):
  * Scaled variables remove all per-step scalar multiplies except 0.8:
      I' = i/s_i, V' = v/(0.2*s_i), with s_i = K_left (0.15).
      c' = c/s_i = u + d + l + r + (8/3)x  (for the given cross kernel)
      V'_dec = 0.8 V' + I';  z = V'_dec > TH (TH = 1/(0.2*s_i));
      V' = (V'_dec <= TH) * V'_dec;  I' = 0.8 I' + c'
    Spike output z is bit-identical (validated in fp32 numpy vs jax ref).
  * Per core: 128 rows on the 128 SBUF partitions, 256 local W cols, T=128.
  * Conv: vertical taps (u + (8/3)c + d) via one fp32 PE matmul with a
    tridiagonal stationary matrix (PE fp32 is exact; fp32r is TF32-like and
    NOT usable). Horizontal taps (l+r) on GPSIMD. H-halo rows added via
    SWDGE DMA-accumulate directly from DRAM. All summed into hsum in SBUF.
  * Synaptic current I': one DVE tensor_tensor_scan per (8w x 128t) slice,
    with a 0.8-multiplier tile whose t=0 slots are 0.0 (per-pixel reset),
    so one scan instruction handles 8 independent pixel recurrences.
  * Membrane V': 127 sequential steps of two scalar_tensor_tensor ops on
    [128, 256]; V_dec overwrites the consumed I' slot in place.
  * Spikes: batched ACT sign -> relu over the stored V_dec values.
"""
import sys

for _p in ("/opt/trn_rl_repo",):
    if _p not in sys.path:
        sys.path.insert(0, _p)

import numpy as np
from concourse import bacc, mybir
import concourse.tile as tile
from concourse.bass_utils import run_bass_kernel_spmd

f32 = mybir.dt.float32

T = 128          # timesteps
RPC = 128        # rows per core (H=512 / 4)
WPC = 256        # cols per core (W=512 / 2)
NWC = 4          # w-chunks per core
WC = WPC // NWC  # 64 cols per chunk
NTH = 4          # t-quarters per chunk DMA
TH_T = T // NTH  # 32
SCAN_W = 16      # w-cols per scan op (F = 16*128 = 2048)
HW_W = 32        # w-cols per hsum tile
ZB = 16          # timesteps per z-output block

_CACHE = {}
_LAST_IN_MAPS = None
TUNE = {"xc_bufs": 3, "tmp_bufs": 2, "ps_bufs": 2}


def _register_const(nc, value, dtype=f32):
    t = nc.alloc_sbuf_tensor(f"const-user-{value}", [128, 1], dtype)
    nc.gpsimd.memset(t.ap(), value)
    nc.const_aps.aps[(dtype, value)] = t.ap()


def _build_cached(s_i, k_up, k_ctr, k_down, k_right):
    key = (s_i, k_up, k_ctr, k_down, k_right)
    if key not in _CACHE:
        _CACHE[key] = _build_with_consts(*key)
    return _CACHE[key]


def _build_with_consts(s_i, k_up, k_ctr, k_down, k_right):
    # activation() with a float bias needs a pre-registered const AP; patch
    # the builder to register -TH right after Bass init.
    TH = 1.0 / (0.2 * s_i)
    nc = bacc.Bacc("TRN2", target_bir_lowering=False, debug=False,
                   num_devices=8)
    _register_const(nc, -TH)
    nc.all_engine_barrier()
    _build_body(nc, s_i, k_up, k_ctr, k_down, k_right)
    if not nc.is_finalized():
        nc.finalize()
    return nc


def _build_body(nc, s_i, k_up, k_ctr, k_down, k_right, ablate=()):
    # identical to _build()'s body after nc creation
    DEC = 0.8
    TH = 1.0 / (0.2 * s_i)

    xm = nc.declare_dram_parameter("xm", [T, RPC, WPC + 2], f32, isOutput=False)
    xh = nc.declare_dram_parameter("xh", [2, WPC + 2, T], f32, isOutput=False)
    wv = nc.declare_dram_parameter("wv", [RPC, RPC], f32, isOutput=False)
    zo = nc.declare_dram_parameter("zo", [T, RPC, WPC], f32, isOutput=True)

    with tile.TileContext(nc) as tc:
        with tc.tile_pool(name="keep", bufs=1) as keep:
            wvt = keep.tile([RPC, RPC], f32)
            nc.scalar.dma_start(wvt[:], wv[:])

            It = keep.tile([128, WPC * T], f32)
            Iv = It[:].rearrange("p (w t) -> p w t", t=T)

            Vt = keep.tile([128, WPC], f32)
            nc.gpsimd.memset(Vt[:], 0.0)

            d0 = keep.tile([128, SCAN_W * T], f32)
            nc.vector.memset(d0[:], DEC)
            d0v = d0[:].rearrange("p (w t) -> p w t", t=T)
            nc.vector.memset(d0v[:, :, 0:1], 0.0)

            TBS = [1, 1, 2, 4, 8] + [16] * 7   # t-block sizes (pipeline priming)
            with tc.tile_pool(name="xc", bufs=TUNE["xc_bufs"]) as xcp, \
                 tc.tile_pool(name="tmp", bufs=TUNE["tmp_bufs"]) as tmpp, \
                 tc.tile_pool(name="ps", bufs=TUNE["ps_bufs"], space="PSUM") as psp:
                dma_engines = [nc.sync, nc.scalar]
                # c-prime accumulates directly in the I buffer (Iv views).
                t0 = 0
                for tb, TB in enumerate(TBS):
                    xc = xcp.tile([128, max(TBS) * (WPC + 2)], f32,
                                  tag="xc", name="xc")[:, :TB * (WPC + 2)]
                    xcv = xc.rearrange("p (t w) -> p t w", w=WPC + 2)
                    dma_engines[tb % 2].dma_start(
                        xcv,
                        xm[t0:t0 + TB, :, :].rearrange("t p w -> p t w"))

                    for s in range(8 if "hsum" not in ablate else 0):
                        ws = s * 32   # local w of this 32-col slice
                        nc.gpsimd.tensor_tensor(
                            Iv[:, ws:ws + 32, t0:t0 + TB],
                            xcv[:, :, ws:ws + 32].rearrange("p t w -> p w t"),
                            xcv[:, :, ws + 2:ws + 34].rearrange("p t w -> p w t"),
                            mybir.AluOpType.add)

                    for q in range(2 if "pe" not in ablate else 0):
                        wq = q * 128
                        pst = psp.tile([128, 2048], f32, tag="pst", name="pst")
                        for m in range(4):
                            wg = wq + m * 32
                            nc.tensor.matmul(
                                pst[:, m * 512:m * 512 + 32 * TB],
                                wvt[:],
                                xcv[:, :, 1 + wg:33 + wg]
                                   .rearrange("p t w -> p w t"),
                                start=True, stop=True)
                        nc.vector.tensor_tensor(
                            Iv[:, wq:wq + 128, t0:t0 + TB],
                            Iv[:, wq:wq + 128, t0:t0 + TB],
                            pst[:].rearrange("p (m c) -> p m c", m=4)
                                [:, :, :32 * TB]
                                .rearrange("p m (w t) -> p m w t", t=TB),
                            mybir.AluOpType.add)
                    t0 += TB

                # H-halo rows into partitions 0 / 127, then scan + copy-back
                for s in range(8):
                    ws = s * 32
                    nc.gpsimd.dma_start(
                        Iv[0:1, ws:ws + 32, :], xh[0:1, 1 + ws:1 + ws + 32, :],
                        accum_op=mybir.AluOpType.add)
                    nc.gpsimd.dma_start(
                        Iv[127:128, ws:ws + 32, :], xh[1:2, 1 + ws:1 + ws + 32, :],
                        accum_op=mybir.AluOpType.add)
                    for k in range(2 if "scan" not in ablate else 0):
                        lo = (ws + k * SCAN_W) * T
                        hi = (ws + (k + 1) * SCAN_W) * T
                        tmp = tmpp.tile([128, SCAN_W * T], f32,
                                        tag="tmp", name="tmp")
                        nc.vector.tensor_tensor_scan(
                            tmp[:], d0[:], It[:, lo:hi],
                            0.0, mybir.AluOpType.mult, mybir.AluOpType.add)
                        nc.scalar.copy(It[:, lo:hi], tmp[:])

            with tc.tile_pool(name="zs", bufs=2) as zsp, \
                 tc.tile_pool(name="sc", bufs=2) as scp:
                z_ends = [16, 32, 48, 64, 80, 96, 112, 120, 124, 126, 128]
                z_sizes = {e: e - s for s, e in
                           zip([0] + z_ends[:-1], z_ends)}
                for t in range(1, T):
                    slot = Iv[:, :, t - 1:t]
                    if "v" not in ablate:
                        nc.vector.scalar_tensor_tensor(
                            slot, Vt[:], DEC, slot,
                            mybir.AluOpType.mult, mybir.AluOpType.add)
                        nc.vector.scalar_tensor_tensor(
                            Vt[:], slot, TH, slot,
                            mybir.AluOpType.is_le, mybir.AluOpType.mult)

                    if "z" not in ablate and (t + 1) in z_sizes:
                        ZBv = z_sizes[t + 1]
                        tb = t + 1 - ZBv
                        zst = zsp.tile([128, ZB * WPC], f32,
                                       tag="zst", name="zst")[:, :ZBv * WPC]
                        zsv = zst.rearrange("p (t w) -> p w t", w=WPC)
                        sct = scp.tile([128, ZB * WPC], f32, tag="sct", name="sct")
                        if tb == 0:
                            nc.gpsimd.memset(zst[:, 0:WPC], 0.0)
                            sin = Iv[:, :, 0:ZBv - 1]
                            sflat = sct[:, 0:(ZBv - 1) * WPC]
                            zout_v = zsv[:, :, 1:ZBv]
                        else:
                            sin = Iv[:, :, tb - 1:tb + ZBv - 1]
                            sflat = sct[:, 0:ZBv * WPC]
                            zout_v = zsv
                        nc.scalar.activation(
                            sflat, sin, mybir.ActivationFunctionType.Sign,
                            bias=-TH, scale=1.0)
                        nc.scalar.activation(
                            zout_v, sflat, mybir.ActivationFunctionType.Relu)
                        nc.sync.dma_start(
                            zo[tb:tb + ZBv].rearrange("t p w -> p t w"),
                            zst.rearrange("p (t w) -> p t w", w=WPC))


def kernel(x, kernel):
    x = np.asarray(x, dtype=np.float32)
    k = np.asarray(kernel, dtype=np.float32)[0, 0]   # [3,3]
    Tn, _, H, W = x.shape
    assert (Tn, H, W) == (T, 512, 512)

    s_i = float(k[1, 0])                  # left tap = 0.15
    k_up, k_ctr, k_down, k_right = (float(k[0, 1]), float(k[1, 1]),
                                    float(k[2, 1]), float(k[1, 2]))

    nc = _build_cached(s_i, k_up, k_ctr, k_down, k_right)

    # vertical tridiagonal stationary matrix (lhsT[p, po]): u/ctr/d taps / s_i
    wvm = np.zeros((RPC, RPC), np.float32)
    cu, cc, cd = np.float32(k_up / s_i), np.float32(k_ctr / s_i), np.float32(k_down / s_i)
    for i in range(RPC):
        wvm[i, i] = cc
        if i + 1 < RPC:
            wvm[i, i + 1] = cu        # input row i feeds output row i+1's up-tap
            wvm[i + 1, i] = cd        # input row i+1 feeds output row i's down-tap
    # check: out[po] = sum_p wvm[p, po] * x[p] = cu*x[po-1] + cc*x[po] + cd*x[po+1]

    xp = np.pad(x[:, 0], ((0, 0), (1, 1), (1, 1)))   # [T, 514, 514]
    in_maps = []
    for c in range(8):
        a, b = divmod(c, 2)
        r0, w0 = 128 * a, 256 * b
        xm = np.ascontiguousarray(xp[:, 1 + r0:1 + r0 + RPC, w0:w0 + WPC + 2])
        top = xp[:, r0, w0 + 1:w0 + 1 + WPC + 2 - 2]        # halo row above, real cols
        bot = xp[:, 1 + r0 + RPC, w0 + 1:w0 + 1 + WPC]
        # xh layout [2, WPC+2, T]; only cols 1..WPC+1 are read (offset 1+w0-w0)
        xh = np.zeros((2, WPC + 2, T), np.float32)
        xh[0, 1:1 + WPC, :] = (top * np.float32(k_up / s_i)).T
        xh[1, 1:1 + WPC, :] = (bot * np.float32(k_down / s_i)).T
        in_maps.append({"xm": xm, "xh": np.ascontiguousarray(xh), "wv": wvm})

    global _LAST_IN_MAPS
    _LAST_IN_MAPS = in_maps
    res = run_bass_kernel_spmd(nc, in_maps, core_ids=list(range(8)))

    out = np.zeros((T, 1, H, W), np.float32)
    for c in range(8):
        a, b = divmod(c, 2)
        out[:, 0, 128 * a:128 * a + RPC, 256 * b:256 * b + WPC] = res.results[c]["zo"]
    return out



# revision 6
# speedup vs baseline: 1.9598x; 1.9598x over previous
"""Trainium2 Bass kernel for nn_Blur1: 3x3 cross blur + LIF neuron scan.

Reference semantics (per timestep t, state v/i per pixel):
    c    = conv2d_same(x[t], K)        # K = cross kernel (0.15 sides, 0.4 ctr)
    v_d  = 0.8*v + 0.2*i
    z[t] = (v_d - 1) > 0
    v    = (1-z)*v_d
    i    = 0.8*i + c

Strategy (8 NeuronCores = 4 H-shards x 2 W-shards, no collectives):
  * Scaled variables with s = k_ctr (0.4): c'' = c/s = 0.375*(u+d+l+r) + x_c,
    I'' = i/s, V'' = v/(0.2*s), TH = 12.5.  All conv coefficients (0.375, 1)
    are exact in fp16.
  * Conv entirely on the PE: per t-chunk of 8, 6 fp16 matmuls accumulate into
    PSUM: a tridiagonal vertical matrix and a 0.375-scaled identity applied
    to w-1 / w+1 shifted views, each against an exact hi/lo fp16 split of x
    (x = x_hi + x_lo, residual ~2^-22 -> 0 spike flips in numpy validation).
    fp16 matmuls run 4x the fp32 rate, so all five taps on PE beat a
    DVE/GPSIMD horizontal pass + merge (GPSIMD cannot access PSUM on TRN2).
  * Synaptic current I'': one tensor_tensor_scan per t-chunk over a
    (w, t_local) slab, multiplier tile 0.8 zeroed at chunk starts; the
    cross-chunk carry is injected by a small STT into the chunk's first
    t-column (reading the slab BEFORE halo accumulation - see below).
    Chunks split between DVE (scans PSUM directly) and GPSIMD (scans an
    ACT-engine copy, since GPSIMD cannot read PSUM).
  * Membrane V'': ONE custom DVE instruction per timestep (fused
    out = select(0.8*V + I < TH, 0.8*V + I, 0)), overwriting the consumed
    I slab slot in place, so slot t-1 ends holding V[t].
  * Spikes: z[t] <=> V[t] == 0 (v_dec == 0 exactly has measure ~0).  ACT
    Sign -> fp8 DMA out; host maps sign==0 to spike; z[0]=0 on host.
  * H-halo rows (partitions 0/127): host precomputes the FULL decayed scan
    of the neighbour-row contribution and SWDGE-accumulates each chunk's
    slice into the I slab after that chunk's scan.  The carry fixup is
    emitted before the previous chunk's halo accum, so the device carry
    chain never includes halo terms (host scan supplies them all) and the
    serial fixup->scan chain never waits on the DMA.
"""
import sys

for _p in ("/opt/trn_rl_repo",):
    if _p not in sys.path:
        sys.path.insert(0, _p)

import numpy as np
from concourse import bacc, mybir
import concourse.tile as tile
from concourse.bass_utils import run_bass_kernel_spmd

f32 = mybir.dt.float32
f16 = mybir.dt.float16
fp8 = mybir.dt.float8e4

T = 128          # timesteps
RPC = 128        # rows per core (H=512 / 4)
WPC = 256        # cols per core (W=512 / 2)
TC = 8           # timesteps per chunk
NCH = T // TC    # 16 chunks
DEC = 0.8
TH = 1.0 / (0.2 * 0.4)   # threshold in k_ctr-scaled units
KS = 0.375               # side tap / center tap

_CACHE = {}
TUNE = {
    "x_bufs": 3, "ps_bufs": 2, "cs_bufs": 2, "zs_bufs": 2,
    # GPSIMD's ISA has no TensorScalarPtr (no STT / no scan), so all scans
    # and carry fixups run on DVE.
    "dve_scan": lambda c: True,
}


def _register_lif_step():
    """LIF_STEP custom DVE op: out = select(y < C0, y, 0), y = Src0*C1 + Src1.
    One DVE instruction per membrane timestep instead of two STT passes."""
    import concourse.dve_ops as dve_ops
    from concourse.dve_spec import (C0, C1, Spec, Src0, Src1, Zero, select,
                                    lower, _has_src1)
    from concourse.dve_uop import DveOpSpec
    from concourse.dve_table_gen import dve_ver_for

    for op in dve_ops.OPS:
        if op.name == "LIF_STEP":
            return op

    y = Src0 * C1 + Src1

    def ref(in0, in1, c0, c1, c2):
        yv = (np.asarray(in0, np.float32) * c1
              + np.asarray(in1, np.float32)).astype(np.float32)
        return np.where(yv < c0, yv, np.float32(0.0)).astype(np.float32)

    spec = Spec(body=select(y < C0, y, Zero), reference=ref)
    name = "LIF_STEP"
    row = max(dve_ops._SUB_OPCODE_FOR_NAME.values()) + 1
    assert row < 0x20
    dve_ops._SUB_OPCODE_FOR_NAME[name] = row
    ver = dve_ver_for("TRN2")
    uops = lower(spec, ver=ver)
    probe = DveOpSpec(name=name, opcode=row, uops=uops, rd1_en=_has_src1(spec))
    op = dve_ops.DveOp(name, spec, subdim=False,
                       uops_sha={ver: probe.sha(ver)})
    dve_ops.OPS.append(op)
    dve_ops.CUSTOM_DVE_SPECS[name] = spec
    return op


def _build_cached():
    if "nc" not in _CACHE:
        _CACHE["nc"] = _build()
    return _CACHE["nc"]


def _build():
    LIF = _register_lif_step()
    nc = bacc.Bacc("TRN2", target_bir_lowering=False, debug=False,
                   num_devices=8)

    xhi = nc.declare_dram_parameter("xhi", [RPC, T, WPC + 2], f16, isOutput=False)
    xlo = nc.declare_dram_parameter("xlo", [RPC, T, WPC + 2], f16, isOutput=False)
    mv = nc.declare_dram_parameter("mv", [RPC, RPC], f16, isOutput=False)
    mh = nc.declare_dram_parameter("mh", [RPC, RPC], f16, isOutput=False)
    xh = nc.declare_dram_parameter("xh", [2, NCH, WPC * TC], f32, isOutput=False)
    zo = nc.declare_dram_parameter("zo", [RPC, T, WPC], fp8, isOutput=True)

    with tile.TileContext(nc) as tc:
        with tc.tile_pool(name="keep", bufs=1) as keep:
            mvt = keep.tile([RPC, RPC], f16)
            mht = keep.tile([RPC, RPC], f16)
            nc.scalar.dma_start(mvt[:], mv[:])
            nc.scalar.dma_start(mht[:], mh[:])

            slab = keep.tile([128, WPC * T], f32)
            slabv = slab[:].rearrange("p (c w t) -> p c w t", w=WPC, t=TC)

            d0 = keep.tile([128, WPC * TC], f32)
            nc.vector.memset(d0[:], DEC)
            d0v = d0[:].rearrange("p (w t) -> p w t", t=TC)
            nc.vector.memset(d0v[:, :, 0:1], 0.0)

            zt = keep.tile([128, WPC], f32)
            nc.vector.memset(zt[:], 0.0)

            def tail(c, zsp):
                """halo accum + V steps + sign + DMA out for chunk c."""
                for hb in (0, 127):
                    nc.gpsimd.dma_start(
                        slabv[hb:hb + 1, c, :, :].rearrange("p w t -> p (w t)"),
                        xh[(0 if hb == 0 else 1):(1 if hb == 0 else 2), c, :],
                        accum_op=mybir.AluOpType.add)
                for t in range(c * TC + 1, c * TC + TC + 1):
                    if t > T - 1:
                        break
                    j = t - 1          # slab slot consumed & overwritten
                    cc, tl = divmod(j, TC)
                    slot = slabv[:, cc, :, tl:tl + 1]
                    if t == 1:
                        in0 = zt[:]
                    else:
                        cp, tp = divmod(j - 1, TC)
                        in0 = slabv[:, cp, :, tp:tp + 1]
                    nc.vector._custom_dve(LIF, out=slot, in0=in0, in1=slot,
                                          s0=TH, s1=DEC)
                zst = zsp.tile([128, WPC * TC], fp8, tag="zst", name="zst")
                zsv = zst[:].rearrange("p (t w) -> p w t", w=WPC)
                nc.scalar.activation(zsv, slabv[:, c, :, :],
                                     mybir.ActivationFunctionType.Sign)
                nc.sync.dma_start(
                    zo[:, c * TC:(c + 1) * TC, :].rearrange("p t w -> p (t w)"),
                    zst[:])

            with tc.tile_pool(name="xb", bufs=TUNE["x_bufs"]) as xbp, \
                 tc.tile_pool(name="ps", bufs=TUNE["ps_bufs"], space="PSUM") as psp, \
                 tc.tile_pool(name="cs", bufs=TUNE["cs_bufs"]) as csp, \
                 tc.tile_pool(name="zs", bufs=TUNE["zs_bufs"]) as zsp:
                for c in range(NCH):
                    t0 = c * TC
                    # ---- front end: load + conv into psum ----
                    xht = xbp.tile([128, TC * (WPC + 2)], f16, tag="xh", name="xht")
                    xlt = xbp.tile([128, TC * (WPC + 2)], f16, tag="xl", name="xlt")
                    nc.sync.dma_start(
                        xht[:], xhi[:, t0:t0 + TC, :].rearrange("p t w -> p (t w)"))
                    nc.sync.dma_start(
                        xlt[:], xlo[:, t0:t0 + TC, :].rearrange("p t w -> p (t w)"))
                    xhv = xht[:].rearrange("p (t w) -> p t w", w=WPC + 2)
                    xlv = xlt[:].rearrange("p (t w) -> p t w", w=WPC + 2)

                    pst = psp.tile([128, WPC * TC], f32, tag="pst", name="pst")
                    pstv = pst[:].rearrange("p (w t) -> p w t", t=TC)
                    plan = [(mvt, xhv, 1), (mvt, xlv, 1),
                            (mht, xhv, 0), (mht, xlv, 0),
                            (mht, xhv, 2), (mht, xlv, 2)]
                    for m in range(4):
                        wg = m * 64
                        out = pst[:, wg * TC:(wg + 64) * TC]
                        for k, (mat, xv, off) in enumerate(plan):
                            nc.tensor.matmul(
                                out, mat[:],
                                xv[:, :, off + wg:off + wg + 64]
                                  .rearrange("p t w -> p w t"),
                                start=(k == 0), stop=(k == len(plan) - 1))

                    # ---- carry fixup + scan: c'' -> I slab (w-major) ----
                    sout = slab[:, c * WPC * TC:(c + 1) * WPC * TC]
                    if TUNE["dve_scan"](c):
                        if c > 0:
                            nc.vector.scalar_tensor_tensor(
                                pstv[:, :, 0:1], slabv[:, c - 1, :, TC - 1:TC],
                                DEC, pstv[:, :, 0:1],
                                mybir.AluOpType.mult, mybir.AluOpType.add)
                        nc.vector.tensor_tensor_scan(
                            sout, d0[:], pst[:], 0.0,
                            mybir.AluOpType.mult, mybir.AluOpType.add)
                    else:
                        cst = csp.tile([128, WPC * TC], f32, tag="cst", name="cst")
                        nc.scalar.copy(cst[:], pst[:])
                        cstv = cst[:].rearrange("p (w t) -> p w t", t=TC)
                        if c > 0:
                            nc.gpsimd.scalar_tensor_tensor(
                                cstv[:, :, 0:1], slabv[:, c - 1, :, TC - 1:TC],
                                DEC, cstv[:, :, 0:1],
                                mybir.AluOpType.mult, mybir.AluOpType.add)
                        nc.gpsimd.tensor_tensor_scan(
                            sout, d0[:], cst[:], 0.0,
                            mybir.AluOpType.mult, mybir.AluOpType.add)

                    # tail of the PREVIOUS chunk: emitted after this chunk's
                    # fixup so the fixup reads the slab before halo accum.
                    if c > 0:
                        tail(c - 1, zsp)
                tail(NCH - 1, zsp)

    if not nc.is_finalized():
        nc.finalize()
    return nc


def kernel(x, kernel):
    x = np.asarray(x, dtype=np.float32)
    k = np.asarray(kernel, dtype=np.float32)[0, 0]   # [3,3]
    Tn, _, H, W = x.shape
    assert (Tn, H, W) == (T, 512, 512)
    s = float(k[1, 1])                    # center tap = 0.4
    assert abs(float(k[1, 0]) / s - KS) < 1e-6

    nc = _build_cached()

    # stationary matrices (fp16-exact entries)
    mvm = np.zeros((RPC, RPC), np.float16)
    for i in range(RPC):
        mvm[i, i] = 1.0
        if i + 1 < RPC:
            mvm[i, i + 1] = KS     # input row i feeds output row i+1's up-tap
            mvm[i + 1, i] = KS     # input row i+1 feeds output row i's down-tap
    mhm = (np.eye(RPC) * KS).astype(np.float16)

    xp = np.pad(x[:, 0], ((0, 0), (1, 1), (1, 1)))   # [T, 514, 514]

    in_maps = []
    for c8 in range(8):
        a, b = divmod(c8, 2)
        r0, w0 = RPC * a, WPC * b
        xs = np.ascontiguousarray(
            xp[:, 1 + r0:1 + r0 + RPC, w0:w0 + WPC + 2].transpose(1, 0, 2))
        xhi_ = xs.astype(np.float16)
        xlo_ = (xs - xhi_.astype(np.float32)).astype(np.float16)

        # full decayed prescan of halo-row contributions (f64 host math)
        top = xp[:, r0, 1 + w0:1 + w0 + WPC].astype(np.float64)        # [T, W]
        bot = xp[:, 1 + r0 + RPC, 1 + w0:1 + w0 + WPC].astype(np.float64)
        xh_ = np.zeros((2, T, WPC), np.float32)
        for hb, row in ((0, top), (1, bot)):
            acc = np.zeros(WPC, np.float64)
            for t in range(T):
                acc = DEC * acc + KS * row[t]
                xh_[hb, t] = acc.astype(np.float32)
        # [2, T, W] -> [2, chunk, w, t_local]
        xh_ = xh_.reshape(2, NCH, TC, WPC).transpose(0, 1, 3, 2)

        in_maps.append({
            "xhi": xhi_, "xlo": xlo_, "mv": mvm, "mh": mhm,
            "xh": np.ascontiguousarray(xh_.reshape(2, NCH, WPC * TC)),
        })

    res = run_bass_kernel_spmd(nc, in_maps, core_ids=list(range(8)))

    out = np.zeros((T, 1, H, W), np.float32)
    for c8 in range(8):
        a, b = divmod(c8, 2)
        s8 = np.asarray(res.results[c8]["zo"]).astype(np.float32)  # [p, t, w]
        zc = np.zeros((T, RPC, WPC), np.float32)
        zc[1:] = (s8[:, 0:T - 1, :] == 0.0).astype(np.float32).transpose(1, 0, 2)
        out[:, 0, RPC * a:RPC * (a + 1), WPC * b:WPC * (b + 1)] = zc
    return out


# revision 22
# speedup vs baseline: 2.0265x; 1.0340x over previous
"""Trainium2 Bass kernel for nn_Blur1: 3x3 cross blur + LIF neuron scan.

Reference semantics (per timestep t, state v/i per pixel):
    c    = conv2d_same(x[t], K)        # K = cross kernel (0.15 sides, 0.4 ctr)
    v_d  = 0.8*v + 0.2*i
    z[t] = (v_d - 1) > 0
    v    = (1-z)*v_d
    i    = 0.8*i + c

Strategy (8 NeuronCores = 4 H-shards x 2 W-shards, no collectives):
  * Scaled variables with s = k_ctr (0.4): c'' = c/s = 0.375*(u+d+l+r) + x_c,
    I'' = i/s, V'' = v/(0.2*s), TH = 12.5.  All conv coefficients (0.375, 1)
    are exact in fp16.
  * Conv entirely on the PE: per t-chunk of 8, 6 fp16 matmuls accumulate into
    PSUM: a tridiagonal vertical matrix and a 0.375-scaled identity applied
    to w-1 / w+1 shifted views, each against an exact hi/lo fp16 split of x
    (x = x_hi + x_lo, residual ~2^-22 -> 0 spike flips in numpy validation).
    fp16 matmuls run 4x the fp32 rate, so all five taps on PE beat a
    DVE/GPSIMD horizontal pass + merge (GPSIMD cannot access PSUM on TRN2).
  * Synaptic current I'': one tensor_tensor_scan per t-chunk over a
    (w, t_local) slab, multiplier tile 0.8 zeroed at chunk starts; the
    cross-chunk carry is injected by a small STT into the chunk's first
    t-column (reading the slab BEFORE halo accumulation - see below).
    Chunks split between DVE (scans PSUM directly) and GPSIMD (scans an
    ACT-engine copy, since GPSIMD cannot read PSUM).
  * Membrane V'': ONE custom DVE instruction per timestep (fused
    out = select(0.8*V + I < TH, 0.8*V + I, 0)), overwriting the consumed
    I slab slot in place, so slot t-1 ends holding V[t].
  * Spikes: z[t] <=> V[t] == 0 (v_dec == 0 exactly has measure ~0).  ACT
    Sign -> fp8 DMA out; host maps sign==0 to spike; z[0]=0 on host.
  * H-halo rows (partitions 0/127): host precomputes the FULL decayed scan
    of the neighbour-row contribution and SWDGE-accumulates each chunk's
    slice into the I slab after that chunk's scan.  The carry fixup is
    emitted before the previous chunk's halo accum, so the device carry
    chain never includes halo terms (host scan supplies them all) and the
    serial fixup->scan chain never waits on the DMA.
"""
import sys

for _p in ("/opt/trn_rl_repo",):
    if _p not in sys.path:
        sys.path.insert(0, _p)

import numpy as np
from concourse import bacc, mybir
import concourse.tile as tile
from concourse.bass_utils import run_bass_kernel_spmd

f32 = mybir.dt.float32
f16 = mybir.dt.float16
fp8 = mybir.dt.float8e4

T = 128          # timesteps
RPC = 128        # rows per core (H=512 / 4)
WPC = 256        # cols per core (W=512 / 2)
# chunk sizes: small first chunks prime the pipeline (scan/V start before the
# PE has ramped), small last chunks shrink the sign/DMA-out tail.
TCS = [2, 2, 4] + [8] * 14 + [4, 2, 2]
assert sum(TCS) == T
OFFS = [sum(TCS[:i]) for i in range(len(TCS))]
NCH = len(TCS)
DEC = 0.8
TH = 1.0 / (0.2 * 0.4)   # threshold in k_ctr-scaled units
KS = 0.375               # side tap / center tap

_CACHE = {}
TUNE = {"x_bufs": 4, "ps_bufs": 2, "zs_bufs": 2}


def _register_lif_step():
    """LIF_STEP custom DVE op: out = select(y < C0, y, 0), y = Src0*C1 + Src1.
    One DVE instruction per membrane timestep instead of two STT passes."""
    import concourse.dve_ops as dve_ops
    from concourse.dve_spec import (C0, C1, Spec, Src0, Src1, Zero, select,
                                    lower, _has_src1)
    from concourse.dve_uop import DveOpSpec
    from concourse.dve_table_gen import dve_ver_for

    for op in dve_ops.OPS:
        if op.name == "LIF_STEP":
            return op

    y = Src0 * C1 + Src1

    def ref(in0, in1, c0, c1, c2):
        yv = (np.asarray(in0, np.float32) * c1
              + np.asarray(in1, np.float32)).astype(np.float32)
        return np.where(yv < c0, yv, np.float32(0.0)).astype(np.float32)

    spec = Spec(body=select(y < C0, y, Zero), reference=ref)
    name = "LIF_STEP"
    row = max(dve_ops._SUB_OPCODE_FOR_NAME.values()) + 1
    assert row < 0x20
    dve_ops._SUB_OPCODE_FOR_NAME[name] = row
    ver = dve_ver_for("TRN2")
    uops = lower(spec, ver=ver)
    probe = DveOpSpec(name=name, opcode=row, uops=uops, rd1_en=_has_src1(spec))
    op = dve_ops.DveOp(name, spec, subdim=False,
                       uops_sha={ver: probe.sha(ver)})
    dve_ops.OPS.append(op)
    dve_ops.CUSTOM_DVE_SPECS[name] = spec
    return op


def _build_cached():
    if "nc" not in _CACHE:
        _CACHE["nc"] = _build()
    return _CACHE["nc"]


def _build():
    LIF = _register_lif_step()
    nc = bacc.Bacc("TRN2", target_bir_lowering=False, debug=False,
                   num_devices=8)

    xhi = nc.declare_dram_parameter("xhi", [RPC, T, WPC + 2], f16, isOutput=False)
    xlo = nc.declare_dram_parameter("xlo", [RPC, T, WPC + 2], f16, isOutput=False)
    mv = nc.declare_dram_parameter("mv", [RPC, RPC], f16, isOutput=False)
    mh = nc.declare_dram_parameter("mh", [RPC, RPC], f16, isOutput=False)
    xh = nc.declare_dram_parameter("xh", [2, T * WPC], f32, isOutput=False)
    zo = nc.declare_dram_parameter("zo", [RPC, T, WPC], fp8, isOutput=True)
    d0d = {tc_: nc.declare_dram_parameter(f"d0_{tc_}", [128, WPC * tc_], f32,
                                          isOutput=False)
           for tc_ in sorted(set(TCS))}
    ztd = nc.declare_dram_parameter("ztd", [128, WPC], f32, isOutput=False)

    with tile.TileContext(nc) as tc:
        with tc.tile_pool(name="keep", bufs=1) as keep:
            mvt = keep.tile([RPC, RPC], f16)
            mht = keep.tile([RPC, RPC], f16)

            slab = keep.tile([128, WPC * T], f32)

            def cview(c):
                """[p, w, t_local] view of chunk c of the slab."""
                tc_, off = TCS[c], OFFS[c]
                return slab[:, off * WPC:(off + tc_) * WPC].rearrange(
                    "p (w t) -> p w t", t=tc_)

            d0s = {tc_: keep.tile([128, WPC * tc_], f32, name=f"d0t{tc_}")
                   for tc_ in sorted(set(TCS))}
            zt = keep.tile([128, WPC], f32)

            with tc.high_priority():
                nc.sync.dma_start(mvt[:], mv[:])
                nc.sync.dma_start(mht[:], mh[:])
                for tc_, d in d0s.items():
                    nc.gpsimd.memset(d[:], DEC)
                    dv = d[:].rearrange("p (w t) -> p w t", t=tc_)
                    nc.gpsimd.memset(dv[:, :, 0:1], 0.0)
                nc.gpsimd.memset(zt[:], 0.0)

            def tail(c, zsp):
                """halo accum + V steps + sign + DMA out for chunk c."""
                tc_, off = TCS[c], OFFS[c]
                cv = cview(c)
                nc.gpsimd.dma_start(
                    cv[0:128:127, :, :].rearrange("p w t -> p (w t)"),
                    xh[:, off * WPC:(off + tc_) * WPC],
                    accum_op=mybir.AluOpType.add)
                for t in range(off + 1, off + tc_ + 1):
                    if t > T - 1:
                        break
                    j = t - 1          # slab slot consumed & overwritten
                    cc = c if j >= off else c - 1
                    slot = cview(cc)[:, :, j - OFFS[cc]:j - OFFS[cc] + 1]
                    if t == 1:
                        in0 = zt[:]
                    else:
                        cp = cc if j - 1 >= OFFS[cc] else cc - 1
                        in0 = cview(cp)[:, :, j - 1 - OFFS[cp]:j - OFFS[cp]]
                    nc.vector._custom_dve(LIF, out=slot, in0=in0, in1=slot,
                                          s0=TH, s1=DEC)
                zst = zsp.tile([128, WPC * max(TCS)], fp8, tag="zst",
                               name="zst")[:, :WPC * tc_]
                zsv = zst.rearrange("p (t w) -> p w t", w=WPC)
                nc.scalar.activation(zsv, cv[:, :, :],
                                     mybir.ActivationFunctionType.Sign)
                nc.sync.dma_start(
                    zo[:, off:off + tc_, :].rearrange("p t w -> p (t w)"),
                    zst)

            with tc.tile_pool(name="xb", bufs=TUNE["x_bufs"]) as xbp, \
                 tc.tile_pool(name="ps", bufs=TUNE["ps_bufs"], space="PSUM") as psp, \
                 tc.tile_pool(name="zs", bufs=TUNE["zs_bufs"]) as zsp:
                for c in range(NCH):
                    tc_, t0 = TCS[c], OFFS[c]
                    # ---- front end: load + conv into psum ----
                    xht = xbp.tile([128, max(TCS) * (WPC + 2)], f16,
                                   tag="xh", name="xht")[:, :tc_ * (WPC + 2)]
                    xlt = xbp.tile([128, max(TCS) * (WPC + 2)], f16,
                                   tag="xl", name="xlt")[:, :tc_ * (WPC + 2)]
                    nc.sync.dma_start(
                        xht, xhi[:, t0:t0 + tc_, :].rearrange("p t w -> p (t w)"))
                    nc.sync.dma_start(
                        xlt, xlo[:, t0:t0 + tc_, :].rearrange("p t w -> p (t w)"))
                    xhv = xht.rearrange("p (t w) -> p t w", w=WPC + 2)
                    xlv = xlt.rearrange("p (t w) -> p t w", w=WPC + 2)

                    pst_t = psp.tile([128, WPC * max(TCS)], f32,
                                     tag="pst", name="pst")
                    pst = pst_t[:, :WPC * tc_]
                    pstv = pst.rearrange("p (w t) -> p w t", t=tc_)
                    plan = [(mvt, xhv, 1), (mvt, xlv, 1),
                            (mht, xhv, 0), (mht, xlv, 0),
                            (mht, xhv, 2), (mht, xlv, 2)]
                    for m in range(4):
                        wg = m * 64
                        out = pst[:, wg * tc_:(wg + 64) * tc_]
                        for k, (mat, xv, off_) in enumerate(plan):
                            nc.tensor.matmul(
                                out, mat[:],
                                xv[:, :, off_ + wg:off_ + wg + 64]
                                  .rearrange("p t w -> p w t"),
                                start=(k == 0), stop=(k == len(plan) - 1))

                    # ---- carry fixup + scan: c'' -> I slab (w-major) ----
                    sout = slab[:, t0 * WPC:(t0 + tc_) * WPC]
                    if c > 0:
                        pt = TCS[c - 1]
                        nc.vector.scalar_tensor_tensor(
                            pstv[:, :, 0:1], cview(c - 1)[:, :, pt - 1:pt],
                            DEC, pstv[:, :, 0:1],
                            mybir.AluOpType.mult, mybir.AluOpType.add)
                    nc.vector.tensor_tensor_scan(
                        sout, d0s[tc_][:], pst, 0.0,
                        mybir.AluOpType.mult, mybir.AluOpType.add)

                    # tail of the PREVIOUS chunk: emitted after this chunk's
                    # fixup so the fixup reads the slab before halo accum.
                    if c > 0:
                        tail(c - 1, zsp)
                tail(NCH - 1, zsp)

    if not nc.is_finalized():
        nc.finalize()
    return nc


def kernel(x, kernel):
    x = np.asarray(x, dtype=np.float32)
    k = np.asarray(kernel, dtype=np.float32)[0, 0]   # [3,3]
    Tn, _, H, W = x.shape
    assert (Tn, H, W) == (T, 512, 512)
    s = float(k[1, 1])                    # center tap = 0.4
    assert abs(float(k[1, 0]) / s - KS) < 1e-6

    nc = _build_cached()

    # stationary matrices (fp16-exact entries)
    mvm = np.zeros((RPC, RPC), np.float16)
    for i in range(RPC):
        mvm[i, i] = 1.0
        if i + 1 < RPC:
            mvm[i, i + 1] = KS     # input row i feeds output row i+1's up-tap
            mvm[i + 1, i] = KS     # input row i+1 feeds output row i's down-tap
    mhm = (np.eye(RPC) * KS).astype(np.float16)

    xp = np.pad(x[:, 0], ((0, 0), (1, 1), (1, 1)))   # [T, 514, 514]

    in_maps = []
    for c8 in range(8):
        a, b = divmod(c8, 2)
        r0, w0 = RPC * a, WPC * b
        xs = np.ascontiguousarray(
            xp[:, 1 + r0:1 + r0 + RPC, w0:w0 + WPC + 2].transpose(1, 0, 2))
        xhi_ = xs.astype(np.float16)
        xlo_ = (xs - xhi_.astype(np.float32)).astype(np.float16)

        # full decayed prescan of halo-row contributions (f64 host math)
        top = xp[:, r0, 1 + w0:1 + w0 + WPC].astype(np.float64)        # [T, W]
        bot = xp[:, 1 + r0 + RPC, 1 + w0:1 + w0 + WPC].astype(np.float64)
        xh_ = np.zeros((2, T, WPC), np.float32)
        for hb, row in ((0, top), (1, bot)):
            acc = np.zeros(WPC, np.float64)
            for t in range(T):
                acc = DEC * acc + KS * row[t]
                xh_[hb, t] = acc.astype(np.float32)
        # pack [2, T, W] into per-chunk (w, t_local) blocks
        xhp = np.zeros((2, T * WPC), np.float32)
        for c, (tc_, off) in enumerate(zip(TCS, OFFS)):
            blk = xh_[:, off:off + tc_, :].transpose(0, 2, 1)   # [2, w, tl]
            xhp[:, off * WPC:(off + tc_) * WPC] = blk.reshape(2, -1)

        im = {"xhi": xhi_, "xlo": xlo_, "mv": mvm, "mh": mhm, "xh": xhp,
              "ztd": np.zeros((128, WPC), np.float32)}
        for tc_ in sorted(set(TCS)):
            d = np.full((128, WPC, tc_), DEC, np.float32)
            d[:, :, 0] = 0.0
            im[f"d0_{tc_}"] = d.reshape(128, WPC * tc_)
        in_maps.append(im)

    res = run_bass_kernel_spmd(nc, in_maps, core_ids=list(range(8)))

    out = np.zeros((T, 1, H, W), np.float32)
    for c8 in range(8):
        a, b = divmod(c8, 2)
        s8 = np.asarray(res.results[c8]["zo"]).astype(np.float32)  # [p, t, w]
        zc = np.zeros((T, RPC, WPC), np.float32)
        zc[1:] = (s8[:, 0:T - 1, :] == 0.0).astype(np.float32).transpose(1, 0, 2)
        out[:, 0, RPC * a:RPC * (a + 1), WPC * b:WPC * (b + 1)] = zc
    return out


# revision 31
# speedup vs baseline: 2.1127x; 1.0425x over previous
"""Trainium2 Bass kernel for nn_Blur1: 3x3 cross blur + LIF neuron scan.

Reference semantics (per timestep t, state v/i per pixel):
    c    = conv2d_same(x[t], K)        # K = cross kernel (0.15 sides, 0.4 ctr)
    v_d  = 0.8*v + 0.2*i
    z[t] = (v_d - 1) > 0
    v    = (1-z)*v_d
    i    = 0.8*i + c

Strategy (8 NeuronCores = 4 H-shards x 2 W-shards, no collectives):
  * Scaled variables with s = k_ctr (0.4): c'' = c/s = 0.375*(u+d+l+r) + x_c,
    I'' = i/s, V'' = v/(0.2*s), TH = 12.5.  All conv coefficients (0.375, 1)
    are exact in fp16.
  * Conv entirely on the PE: per t-chunk of 8, 6 fp16 matmuls accumulate into
    PSUM: a tridiagonal vertical matrix and a 0.375-scaled identity applied
    to w-1 / w+1 shifted views, each against an exact hi/lo fp16 split of x
    (x = x_hi + x_lo, residual ~2^-22 -> 0 spike flips in numpy validation).
    fp16 matmuls run 4x the fp32 rate, so all five taps on PE beat a
    DVE/GPSIMD horizontal pass + merge (GPSIMD cannot access PSUM on TRN2).
  * Synaptic current I'': one tensor_tensor_scan per t-chunk over a
    (w, t_local) slab, multiplier tile 0.8 zeroed at chunk starts; the
    cross-chunk carry is injected by a small STT into the chunk's first
    t-column (reading the slab BEFORE halo accumulation - see below).
    Chunks split between DVE (scans PSUM directly) and GPSIMD (scans an
    ACT-engine copy, since GPSIMD cannot read PSUM).
  * Membrane V'': ONE custom DVE instruction per timestep (fused
    out = select(0.8*V + I < TH, 0.8*V + I, 0)), overwriting the consumed
    I slab slot in place, so slot t-1 ends holding V[t].
  * Spikes: z[t] <=> V[t] == 0 (v_dec == 0 exactly has measure ~0).  ACT
    Sign -> fp8 DMA out; host maps sign==0 to spike; z[0]=0 on host.
  * H-halo rows (partitions 0/127): host precomputes the FULL decayed scan
    of the neighbour-row contribution and SWDGE-accumulates each chunk's
    slice into the I slab after that chunk's scan.  The carry fixup is
    emitted before the previous chunk's halo accum, so the device carry
    chain never includes halo terms (host scan supplies them all) and the
    serial fixup->scan chain never waits on the DMA.
"""
import sys

for _p in ("/opt/trn_rl_repo",):
    if _p not in sys.path:
        sys.path.insert(0, _p)

import numpy as np
from concourse import bacc, mybir
import concourse.tile as tile
from concourse.bass_utils import run_bass_kernel_spmd

f32 = mybir.dt.float32
f16 = mybir.dt.float16
fp8 = mybir.dt.float8e4

T = 128          # timesteps
RPC = 128        # rows per core (H=512 / 4)
WPC = 256        # cols per core (W=512 / 2)
# chunk sizes: small first chunks prime the pipeline (scan/V start before the
# PE has ramped), small last chunks shrink the sign/DMA-out tail.
TCS = [4, 4] + [8] * 14 + [4, 4]
assert sum(TCS) == T
OFFS = [sum(TCS[:i]) for i in range(len(TCS))]
NCH = len(TCS)
DEC = 0.8
TH = 1.0 / (0.2 * 0.4)   # threshold in k_ctr-scaled units
KS = 0.375               # side tap / center tap

_CACHE = {}
TUNE = {"x_bufs": 3, "ps_bufs": 2, "zs_bufs": 4}


def _register_lif_step():
    """LIF_STEP custom DVE op: out = select(y < C0, y, 0), y = Src0*C1 + Src1.
    One DVE instruction per membrane timestep instead of two STT passes."""
    import concourse.dve_ops as dve_ops
    from concourse.dve_spec import (C0, C1, Spec, Src0, Src1, Zero, select,
                                    lower, _has_src1)
    from concourse.dve_uop import DveOpSpec
    from concourse.dve_table_gen import dve_ver_for

    for op in dve_ops.OPS:
        if op.name == "LIF_STEP":
            return op

    y = Src0 * C1 + Src1

    def ref(in0, in1, c0, c1, c2):
        yv = (np.asarray(in0, np.float32) * c1
              + np.asarray(in1, np.float32)).astype(np.float32)
        return np.where(yv < c0, yv, np.float32(0.0)).astype(np.float32)

    spec = Spec(body=select(y < C0, y, Zero), reference=ref)
    name = "LIF_STEP"
    row = max(dve_ops._SUB_OPCODE_FOR_NAME.values()) + 1
    assert row < 0x20
    dve_ops._SUB_OPCODE_FOR_NAME[name] = row
    ver = dve_ver_for("TRN2")
    uops = lower(spec, ver=ver)
    probe = DveOpSpec(name=name, opcode=row, uops=uops, rd1_en=_has_src1(spec))
    op = dve_ops.DveOp(name, spec, subdim=False,
                       uops_sha={ver: probe.sha(ver)})
    dve_ops.OPS.append(op)
    dve_ops.CUSTOM_DVE_SPECS[name] = spec
    return op


def _build_cached():
    if "nc" not in _CACHE:
        _CACHE["nc"] = _build()
    return _CACHE["nc"]


def _build():
    LIF = _register_lif_step()
    nc = bacc.Bacc("TRN2", target_bir_lowering=False, debug=False,
                   num_devices=8)

    xhi = nc.declare_dram_parameter("xhi", [RPC, T, WPC + 2], f16, isOutput=False)
    xlo = nc.declare_dram_parameter("xlo", [RPC, T, WPC + 2], f16, isOutput=False)
    mv = nc.declare_dram_parameter("mv", [RPC, RPC], f16, isOutput=False)
    mh = nc.declare_dram_parameter("mh", [RPC, RPC], f16, isOutput=False)
    xh = nc.declare_dram_parameter("xh", [2, T * WPC], f32, isOutput=False)
    zo = nc.declare_dram_parameter("zo", [RPC, T, WPC], fp8, isOutput=True)
    d0d = {tc_: nc.declare_dram_parameter(f"d0_{tc_}", [128, WPC * tc_], f32,
                                          isOutput=False)
           for tc_ in sorted(set(TCS))}
    ztd = nc.declare_dram_parameter("ztd", [128, WPC], f32, isOutput=False)

    with tile.TileContext(nc) as tc:
        with tc.tile_pool(name="keep", bufs=1) as keep:
            mvt = keep.tile([RPC, RPC], f16)
            mht = keep.tile([RPC, RPC], f16)

            slab = keep.tile([128, WPC * T], f32)

            def cview(c):
                """[p, w, t_local] view of chunk c of the slab."""
                tc_, off = TCS[c], OFFS[c]
                return slab[:, off * WPC:(off + tc_) * WPC].rearrange(
                    "p (w t) -> p w t", t=tc_)

            d0s = {tc_: keep.tile([128, WPC * tc_], f32, name=f"d0t{tc_}")
                   for tc_ in sorted(set(TCS))}
            zt = keep.tile([128, WPC], f32)

            with tc.high_priority():
                nc.sync.dma_start(mvt[:], mv[:])
                nc.sync.dma_start(mht[:], mh[:])
                for tc_, d in d0s.items():
                    nc.gpsimd.memset(d[:], DEC)
                    dv = d[:].rearrange("p (w t) -> p w t", t=tc_)
                    nc.gpsimd.memset(dv[:, :, 0:1], 0.0)
                nc.gpsimd.memset(zt[:], 0.0)

            # PE warmup: keep the tensor engine busy from t~0 so the first
            # real conv matmuls run at full pstate (ramp needs ~3us busy).
            with tc.tile_pool(name="wu", bufs=1, space="PSUM") as wup:
                wut = wup.tile([128, 128], f32)
                with tc.high_priority():
                    for _ in range(6):
                        nc.tensor.matmul(wut[:], mvt[:], mht[:],
                                         start=True, stop=True)

            def halo(c):
                """accumulate host-prescanned halo rows into slab chunk c."""
                tc_, off = TCS[c], OFFS[c]
                nc.gpsimd.dma_start(
                    cview(c)[0:128:127, :, :].rearrange("p w t -> p (w t)"),
                    xh[:, off * WPC:(off + tc_) * WPC],
                    accum_op=mybir.AluOpType.add)

            def tail(c, zsp):
                """V steps + sign + DMA out for chunk c."""
                tc_, off = TCS[c], OFFS[c]
                cv = cview(c)
                for t in range(off + 1, off + tc_ + 1):
                    if t > T - 1:
                        break
                    j = t - 1          # slab slot consumed & overwritten
                    cc = c if j >= off else c - 1
                    slot = cview(cc)[:, :, j - OFFS[cc]:j - OFFS[cc] + 1]
                    if t == 1:
                        in0 = zt[:]
                    else:
                        cp = cc if j - 1 >= OFFS[cc] else cc - 1
                        in0 = cview(cp)[:, :, j - 1 - OFFS[cp]:j - OFFS[cp]]
                    nc.vector._custom_dve(LIF, out=slot, in0=in0, in1=slot,
                                          s0=TH, s1=DEC)
                zst = zsp.tile([128, WPC * max(TCS)], fp8, tag="zst",
                               name="zst")[:, :WPC * tc_]
                zsv = zst.rearrange("p (t w) -> p w t", w=WPC)
                nc.scalar.activation(zsv, cv[:, :, :],
                                     mybir.ActivationFunctionType.Sign)
                nc.sync.dma_start(
                    zo[:, off:off + tc_, :].rearrange("p t w -> p (t w)"),
                    zst)

            with tc.tile_pool(name="xb", bufs=TUNE["x_bufs"]) as xbp, \
                 tc.tile_pool(name="ps", bufs=TUNE["ps_bufs"], space="PSUM") as psp, \
                 tc.tile_pool(name="zs", bufs=TUNE["zs_bufs"]) as zsp:
                for c in range(NCH):
                    tc_, t0 = TCS[c], OFFS[c]
                    # ---- front end: load + conv into psum ----
                    xht = xbp.tile([128, max(TCS) * (WPC + 2)], f16,
                                   tag="xh", name="xht")[:, :tc_ * (WPC + 2)]
                    xlt = xbp.tile([128, max(TCS) * (WPC + 2)], f16,
                                   tag="xl", name="xlt")[:, :tc_ * (WPC + 2)]
                    nc.sync.dma_start(
                        xht, xhi[:, t0:t0 + tc_, :].rearrange("p t w -> p (t w)"))
                    nc.sync.dma_start(
                        xlt, xlo[:, t0:t0 + tc_, :].rearrange("p t w -> p (t w)"))
                    xhv = xht.rearrange("p (t w) -> p t w", w=WPC + 2)
                    xlv = xlt.rearrange("p (t w) -> p t w", w=WPC + 2)

                    pst_t = psp.tile([128, WPC * max(TCS)], f32,
                                     tag="pst", name="pst")
                    pst = pst_t[:, :WPC * tc_]
                    pstv = pst.rearrange("p (w t) -> p w t", t=tc_)
                    plan = [(mvt, xhv, 1), (mvt, xlv, 1),
                            (mht, xhv, 0), (mht, xlv, 0),
                            (mht, xhv, 2), (mht, xlv, 2)]
                    for m in range(4):
                        wg = m * 64
                        out = pst[:, wg * tc_:(wg + 64) * tc_]
                        for k, (mat, xv, off_) in enumerate(plan):
                            nc.tensor.matmul(
                                out, mat[:],
                                xv[:, :, off_ + wg:off_ + wg + 64]
                                  .rearrange("p t w -> p w t"),
                                start=(k == 0), stop=(k == len(plan) - 1))

                    # ---- carry fixup + scan: c'' -> I slab (w-major) ----
                    sout = slab[:, t0 * WPC:(t0 + tc_) * WPC]
                    if c > 0:
                        pt = TCS[c - 1]
                        nc.vector.scalar_tensor_tensor(
                            pstv[:, :, 0:1], cview(c - 1)[:, :, pt - 1:pt],
                            DEC, pstv[:, :, 0:1],
                            mybir.AluOpType.mult, mybir.AluOpType.add)
                    # halo for the previous chunk: after the fixup's pre-halo
                    # read of the slab, with ~2 chunk-periods before tail(c-1)
                    # consumes it, so the SWDGE latency stays off the V chain.
                    if c > 0:
                        halo(c - 1)
                    nc.vector.tensor_tensor_scan(
                        sout, d0s[tc_][:], pst, 0.0,
                        mybir.AluOpType.mult, mybir.AluOpType.add)

                    if c > 1:
                        tail(c - 2, zsp)
                halo(NCH - 1)
                tail(NCH - 2, zsp)
                tail(NCH - 1, zsp)

    if not nc.is_finalized():
        nc.finalize()
    return nc


def kernel(x, kernel):
    x = np.asarray(x, dtype=np.float32)
    k = np.asarray(kernel, dtype=np.float32)[0, 0]   # [3,3]
    Tn, _, H, W = x.shape
    assert (Tn, H, W) == (T, 512, 512)
    s = float(k[1, 1])                    # center tap = 0.4
    assert abs(float(k[1, 0]) / s - KS) < 1e-6

    nc = _build_cached()

    # stationary matrices (fp16-exact entries)
    mvm = np.zeros((RPC, RPC), np.float16)
    for i in range(RPC):
        mvm[i, i] = 1.0
        if i + 1 < RPC:
            mvm[i, i + 1] = KS     # input row i feeds output row i+1's up-tap
            mvm[i + 1, i] = KS     # input row i+1 feeds output row i's down-tap
    mhm = (np.eye(RPC) * KS).astype(np.float16)

    xp = np.pad(x[:, 0], ((0, 0), (1, 1), (1, 1)))   # [T, 514, 514]

    in_maps = []
    for c8 in range(8):
        a, b = divmod(c8, 2)
        r0, w0 = RPC * a, WPC * b
        xs = np.ascontiguousarray(
            xp[:, 1 + r0:1 + r0 + RPC, w0:w0 + WPC + 2].transpose(1, 0, 2))
        xhi_ = xs.astype(np.float16)
        xlo_ = (xs - xhi_.astype(np.float32)).astype(np.float16)

        # full decayed prescan of halo-row contributions (f64 host math)
        top = xp[:, r0, 1 + w0:1 + w0 + WPC].astype(np.float64)        # [T, W]
        bot = xp[:, 1 + r0 + RPC, 1 + w0:1 + w0 + WPC].astype(np.float64)
        xh_ = np.zeros((2, T, WPC), np.float32)
        for hb, row in ((0, top), (1, bot)):
            acc = np.zeros(WPC, np.float64)
            for t in range(T):
                acc = DEC * acc + KS * row[t]
                xh_[hb, t] = acc.astype(np.float32)
        # pack [2, T, W] into per-chunk (w, t_local) blocks
        xhp = np.zeros((2, T * WPC), np.float32)
        for c, (tc_, off) in enumerate(zip(TCS, OFFS)):
            blk = xh_[:, off:off + tc_, :].transpose(0, 2, 1)   # [2, w, tl]
            xhp[:, off * WPC:(off + tc_) * WPC] = blk.reshape(2, -1)

        im = {"xhi": xhi_, "xlo": xlo_, "mv": mvm, "mh": mhm, "xh": xhp,
              "ztd": np.zeros((128, WPC), np.float32)}
        for tc_ in sorted(set(TCS)):
            d = np.full((128, WPC, tc_), DEC, np.float32)
            d[:, :, 0] = 0.0
            im[f"d0_{tc_}"] = d.reshape(128, WPC * tc_)
        in_maps.append(im)

    res = run_bass_kernel_spmd(nc, in_maps, core_ids=list(range(8)))

    out = np.zeros((T, 1, H, W), np.float32)
    for c8 in range(8):
        a, b = divmod(c8, 2)
        s8 = np.asarray(res.results[c8]["zo"]).astype(np.float32)  # [p, t, w]
        zc = np.zeros((T, RPC, WPC), np.float32)
        zc[1:] = (s8[:, 0:T - 1, :] == 0.0).astype(np.float32).transpose(1, 0, 2)
        out[:, 0, RPC * a:RPC * (a + 1), WPC * b:WPC * (b + 1)] = zc
    return out
